# revision 4
# baseline (speedup 1.0000x reference)
"""GQA causal attention (B=2, S=2048, HID=2048, H=32, HKV=8, D=128) on 8 TRN2
NeuronCores.

Sharding: tensor-parallel over heads for QKV+attention (core c owns kv head c
and q heads 4c..4c+3), then an AllToAll switches to sequence-parallel for
o_proj (core c computes the full hidden dim for global s-chunk c). The A2A
moves 8x less data than an AllGather and needs no per-core dynamic slicing.
It is split into two collectives (head pairs) so comm overlaps attention
compute of the remaining heads and the first half of o_proj.

Device pipeline (bf16 compute, fp32 PSUM accumulation):
  1. Feature-major projections: Q^T/K^T/V^T = W^T h^T, h^T streamed. All
     HBM-resident operands are host-pretiled so every DMA is >=2KB-contiguous
     per partition (h^T chunks 16KB, weights 4-16KB) - the 1KB-line layouts
     capped DMA at ~230GB/s and starved the PE ramp.
  2. RoPE as  x*cos_dup + swap_halves(x)*sin_signed  - the rotate-half is a
     pure partition swap done by idle gpsimd SWDGE DMAs (the sign lives in the
     host-prepared sin table); cross-partition DVE ops are illegal.
  3. Transposed flash attention processed in PAIRS of 128-k-tiles: each score
     pair is one [128,1024] 2-bank PSUM tile (two matmuls), one 1024-wide exp
     on ScalarE (halves ACT instruction overhead - ACT is the binding engine
     in this phase), causal 0/1 mask on diagonal tiles, denominator via
     ones-matmul on pair-sums (quad = add of two pairs: one 1024-wide + one
     512-wide DVE add instead of three 512 adds), out^T += V_tile.T @ P^T.
  4. Two AllToAlls (heads 0-1, then 2-3) exchange attn-out^T blocks.
  5. o_proj: out^T[hid, my_s_chunk] accumulated over all 32 feature tiles
     (Wo host-pretiled into [p][half][hid_t][ft][c] so each wo_t DMA is
     4KB-contiguous), fp32 out.
Host reassembles the 8 sequence chunks and transposes back.
"""

import os

import numpy as np
import ml_dtypes

from concourse import bacc, mybir
import concourse.tile as tile
from concourse.bass_utils import run_bass_kernel_spmd

N_CORES = 8
B, S, HID = 2, 2048, 2048
H, HKV, D = 32, 8, 128
QH = H // HKV          # q heads per core
SG = B * S             # 4096 global sequence
NSC = SG // 512        # 8 s-chunks of 512
NKT = HID // 128       # 16 hid k-tiles
NFT = (H * D) // 128   # 32 o_proj contraction tiles

BF = mybir.dt.bfloat16
F32 = mybir.dt.float32
AF = mybir.ActivationFunctionType

_CACHE = {}
LAST_EXEC_NS = None
LAST_RESULT = None


def _build():
    nc = bacc.Bacc("TRN2", num_devices=N_CORES)

    # Host-pretiled layouts: partition dim first, per-partition runs contiguous.
    hT_e = nc.declare_dram_parameter("hT", [128, NSC * NKT * 512], BF, isOutput=False)
    wq_e = nc.declare_dram_parameter("wq", [128, NKT * 512], BF, isOutput=False)
    wk_e = nc.declare_dram_parameter("wk", [128, NKT * D], BF, isOutput=False)
    wv_e = nc.declare_dram_parameter("wv", [128, NKT * D], BF, isOutput=False)
    wo_e = nc.declare_dram_parameter("wo", [128, 2 * NKT * 16 * 128], BF, isOutput=False)
    cd_e = nc.declare_dram_parameter("c_dup", [D, SG], BF, isOutput=False)
    sd_e = nc.declare_dram_parameter("s_dup", [D, SG], BF, isOutput=False)
    id_e = nc.declare_dram_parameter("ident", [D, D], BF, isOutput=False)
    mk_e = nc.declare_dram_parameter("masks", [128, 1024], BF, isOutput=False)
    outT_e = nc.declare_dram_parameter("outT", [HID, 512], F32, isOutput=True)

    hT_r = hT_e[:].rearrange("p (sc kt s) -> p sc kt s", sc=NSC, kt=NKT)
    wq_r = wq_e[:].rearrange("p (kt f) -> p kt f", kt=NKT)
    wo_r = wo_e[:].rearrange("p (hf ht ft c) -> p hf ht ft c", hf=2, ht=NKT, ft=16)

    with tile.TileContext(nc) as tc:
        with (
            tc.tile_pool(name="cst", bufs=1) as cst,
            tc.tile_pool(name="sb", bufs=2) as sb,
            tc.tile_pool(name="ps", bufs=2, space="PSUM") as ps,
            tc.tile_pool(name="psacc", bufs=2, space="PSUM") as psacc,
            tc.tile_pool(name="dram", bufs=1, space="DRAM") as dram,
        ):
            tril = cst.tile([128, 1024], BF, tag="tril")
            nc.sync.dma_start(tril[:], mk_e[:])
            ones_mat = cst.tile([128, 128], BF, tag="ones_mat")
            nc.gpsimd.memset(ones_mat[:], 1.0)

            qr = cst.tile([128, QH * SG], BF, tag="qr")
            kr = cst.tile([128, SG], BF, tag="kr")
            v_seq = cst.tile([128, SG], BF, tag="v_seq")

            # A2A bounce buffers: shard j = rows [j*256, (j+1)*256) =
            # (2 heads x 128d, s-chunk j's 512 cols).
            a2a_in = [
                dram.tile([8 * 256, 512], BF, name=f"a2ain{i}", tag=f"a2ain{i}")
                for i in (0, 1)
            ]
            a2a_out = [
                dram.tile([8 * 256, 512], BF, name=f"a2aout{i}", tag=f"a2aout{i}")
                for i in (0, 1)
            ]

            # ---- phase 1: projections + rope + V transpose ----
            with tc.tile_pool(name="p1", bufs=1) as p1, \
                 tc.tile_pool(name="htp", bufs=3) as htp:
                # wq and the first h^T chunk interleaved FIRST (4 pieces each,
                # 4KB runs) so the first projection chain streams as pieces
                # land; everything else follows in need-order.
                wq_sb = p1.tile([128, NKT, QH * D], BF, tag="wq_sb")
                ht0 = htp.tile([128, NKT, 512], BF, tag="ht")
                for kq in range(4):
                    nc.sync.dma_start(
                        wq_sb[:, 4 * kq : 4 * kq + 4, :],
                        wq_r[:, 4 * kq : 4 * kq + 4, :],
                    )
                    nc.sync.dma_start(
                        ht0[:, 4 * kq : 4 * kq + 4, :],
                        hT_r[:, 0, 4 * kq : 4 * kq + 4, :],
                    )
                ident = p1.tile([D, D], BF, tag="ident")
                nc.sync.dma_start(ident[:], id_e[:])
                wk_sb = p1.tile([128, NKT, D], BF, tag="wk_sb")
                nc.sync.dma_start(
                    wk_sb[:], wk_e[:].rearrange("p (kt f) -> p kt f", kt=NKT)
                )
                wv_sb = p1.tile([128, NKT, D], BF, tag="wv_sb")
                nc.sync.dma_start(
                    wv_sb[:], wv_e[:].rearrange("p (kt f) -> p kt f", kt=NKT)
                )
                ht1 = htp.tile([128, NKT, 512], BF, tag="ht")
                nc.sync.dma_start(ht1[:], hT_r[:, 1, :, :])
                c_d = p1.tile([D, SG], BF, tag="c_d")
                nc.sync.dma_start(c_d[:], cd_e[:])
                s_d = p1.tile([D, SG], BF, tag="s_d")
                nc.sync.dma_start(s_d[:], sd_e[:])

                # rope/V-transpose for tile i are emitted AFTER projection
                # chain i+1 so their PE ops never wait on the ACT evacuation.
                def finish_tile(sc, ft, xb):
                    if ft < QH + 1:  # rope for q heads and k
                        # rotate-half = partition swap via idle gpsimd SWDGE
                        # (sin table sign-folded on host)
                        sh = sb.tile([128, 512], BF, tag="sh", bufs=3)
                        nc.gpsimd.dma_start(sh[0:64, :], xb[64:128, :])
                        nc.gpsimd.dma_start(sh[64:128, :], xb[0:64, :])
                        if ft < QH:
                            dest = qr[
                                :, ft * SG + sc * 512 : ft * SG + sc * 512 + 512
                            ]
                        else:
                            dest = kr[:, sc * 512 : sc * 512 + 512]
                        cs = c_d[:, sc * 512 : (sc + 1) * 512]
                        ss = s_d[:, sc * 512 : (sc + 1) * 512]
                        nc.vector.tensor_mul(dest, xb[:], cs)
                        rtmp = sb.tile([128, 512], BF, tag="rtmp")
                        nc.vector.tensor_mul(rtmp[:], sh[:], ss)
                        nc.vector.tensor_add(dest, dest, rtmp[:])
                    else:  # v: transpose to seq-major
                        for j in range(4):
                            tp = ps.tile([128, 1024], BF, tag="mmp")
                            nc.tensor.transpose(
                                tp[:, 0:128], xb[:, j * 128 : (j + 1) * 128],
                                ident[:],
                            )
                            g = sc * 4 + j
                            nc.vector.tensor_copy(
                                v_seq[:, g * 128 : (g + 1) * 128], tp[:, 0:128]
                            )

                with nc.named_scope("proj"):
                    pending = None
                    for sc in range(NSC):
                        if sc == 0:
                            ht = ht0
                        elif sc == 1:
                            ht = ht1
                        else:
                            ht = htp.tile([128, NKT, 512], BF, tag="ht")
                            nc.sync.dma_start(ht[:], hT_r[:, sc, :, :])
                        for ft in range(QH + 2):  # 0..3 q heads, 4 k, 5 v
                            acc = ps.tile([128, 512], F32, tag="mm")
                            for kt in range(NKT):
                                if ft < QH:
                                    lhsT = wq_sb[:, kt, ft * D : (ft + 1) * D]
                                elif ft == QH:
                                    lhsT = wk_sb[:, kt, :]
                                else:
                                    lhsT = wv_sb[:, kt, :]
                                nc.tensor.matmul(
                                    acc[:], lhsT, ht[:, kt, :],
                                    start=(kt == 0), stop=(kt == NKT - 1),
                                )
                            xb = sb.tile([128, 512], BF, tag="xb", bufs=4)
                            nc.scalar.activation(xb[:], acc[:], AF.Copy)
                            if pending is not None:
                                finish_tile(*pending)
                            pending = (sc, ft, xb)
                    finish_tile(*pending)

            # ---- phase 2: attention (h outer so A2A can fire per head-pair)
            def attn_head(h, b, qc):
                nkt = 4 * qc + 4
                npair = nkt // 2
                acc = psacc.tile([128, 512], F32, tag="acc")
                den = ps.tile([128, 512], F32, tag="mm")
                qs = h * SG + b * S + qc * 512

                # diagonal tile j (= kt - 4qc >= 0) only contributes to
                # q >= j*128: truncate its q range to [j*128, 512).
                def qoff(kt):
                    j = kt - 4 * qc
                    return j * 128 if j > 0 else 0

                def score_pair(p):
                    s_ps = ps.tile(
                        [128, 1024], F32, tag="mmp", name=f"s_{h}_{b}_{qc}_{p}"
                    )
                    for hf in range(2):
                        kt = 2 * p + hf
                        o = qoff(kt)
                        nc.tensor.matmul(
                            s_ps[:, hf * 512 + o : (hf + 1) * 512],
                            kr[:, b * S + kt * 128 : b * S + (kt + 1) * 128],
                            qr[:, qs + o : qs + 512],
                        )
                    return s_ps

                pipe = [score_pair(p) for p in range(min(2, npair))]
                hold = []
                for p in range(npair):
                    if p + 2 < npair:
                        pipe.append(score_pair(p + 2))
                    s_ps = pipe.pop(0)
                    offdiag = 2 * p + 1 < 4 * qc  # both halves off-diagonal
                    pT = sb.tile([128, 1024], BF, tag="pT", bufs=4)
                    if offdiag:
                        nc.scalar.activation(pT[:], s_ps[:], AF.Exp)
                    else:
                        for hf in range(2):
                            kt = 2 * p + hf
                            o = qoff(kt)
                            sl = slice(hf * 512 + o, (hf + 1) * 512)
                            nc.scalar.activation(pT[:, sl], s_ps[:, sl], AF.Exp)
                            if kt - 4 * qc >= 0:
                                nc.vector.tensor_mul(
                                    pT[:, sl], pT[:, sl], tril[:, : 512 - o]
                                )
                    for hf in range(2):
                        kt = 2 * p + hf
                        o = qoff(kt)
                        g = b * 16 + kt
                        nc.tensor.matmul(
                            acc[:, o:512],
                            v_seq[:, g * 128 : (g + 1) * 128],
                            pT[:, hf * 512 + o : (hf + 1) * 512],
                            start=(kt == 0), stop=(kt == nkt - 1),
                        )
                    if offdiag:
                        # den is linear: quad = pair + pair via one 1024-wide
                        # and one 512-wide DVE add -> 1 den matmul per 4 tiles.
                        hold.append(pT)
                        if len(hold) == 2:
                            quad = sb.tile([128, 1024], BF, tag="quad", bufs=2)
                            nc.vector.tensor_add(quad[:], hold[0][:], hold[1][:])
                            qsum = sb.tile([128, 512], BF, tag="qsum", bufs=2)
                            nc.vector.tensor_add(
                                qsum[:], quad[:, 0:512], quad[:, 512:1024]
                            )
                            nc.tensor.matmul(
                                den[:], ones_mat[:], qsum[:],
                                start=(p == 1), stop=False,
                            )
                            hold = []
                    else:
                        for hf in range(2):
                            kt = 2 * p + hf
                            o = qoff(kt)
                            nc.tensor.matmul(
                                den[:, o:512],
                                ones_mat[:],
                                pT[:, hf * 512 + o : (hf + 1) * 512],
                                start=(kt == 0), stop=(kt == nkt - 1),
                            )
                # den rows are identical (all-ones stationary) == denominator
                # already broadcast across partitions.
                rb_sb = sb.tile([128, 512], F32, tag="rb_sb")
                nc.vector.reciprocal_approx_fast(rb_sb[:], den[:])
                ao = sb.tile([128, 512], BF, tag="ao", bufs=3)
                nc.vector.tensor_mul(ao[:], acc[:], rb_sb[:])
                half, hh = divmod(h, 2)
                sc = b * 4 + qc
                nc.sync.dma_start(
                    a2a_in[half][sc * 256 + hh * 128 : sc * 256 + (hh + 1) * 128, :],
                    ao[:],
                )

            with nc.named_scope("attn"):
                for half in range(2):
                    for h in (2 * half, 2 * half + 1):
                        for b in range(B):
                            for qc in range(4):
                                attn_head(h, b, qc)
                    nc.gpsimd.collective_compute(
                        "AllToAll",
                        mybir.AluOpType.bypass,
                        replica_groups=[list(range(N_CORES))],
                        ins=[a2a_in[half].opt()],
                        outs=[a2a_out[half].opt()],
                    )

            # ---- phase 4: o_proj for my s-chunk, all hidden columns.
            # Two passes: pass 0 (features from A2A1) accumulates to SBUF
            # partials while A2A2 is still in flight; pass 1 adds the rest.
            with nc.named_scope("oproj"), \
                 tc.tile_pool(name="agp", bufs=1) as agp, \
                 tc.tile_pool(name="wop", bufs=8) as wop, \
                 tc.tile_pool(name="prt", bufs=1) as prt:
                parts = []
                for half in range(2):
                    agt = agp.tile([128, 16, 512], BF, tag=f"ag{half}")
                    # gpsimd (SWDGE) queue: serialized behind the collective
                    # wait anyway - keeps this collective-gated load off the
                    # SP HWDGE queue. 4 coarse pieces instead of 16 singles
                    # so the first o_proj chain isn't gated on SWDGE dispatch.
                    for fq in range(4):
                        nc.gpsimd.dma_start(
                            agt[:, 4 * fq : 4 * fq + 4, :],
                            a2a_out[half][
                                4 * fq * 128 : (4 * fq + 4) * 128, :
                            ].rearrange("(ft p) s -> p ft s", p=128),
                        )
                    for hid_t in range(NKT):  # 16 tiles of 128 hidden cols
                        wo_t = wop.tile([128, 16, 128], BF, tag="wo_t")
                        nc.scalar.dma_start(wo_t[:], wo_r[:, half, hid_t, :, :])
                        o_ps = ps.tile([128, 512], F32, tag="mm")
                        for ft in range(16):
                            nc.tensor.matmul(
                                o_ps[:],
                                wo_t[:, ft, :],
                                agt[:, ft, :],
                                start=(ft == 0),
                                stop=(ft == 15),
                            )
                        if half == 0:
                            part = prt.tile(
                                [128, 512], F32, tag=f"part{hid_t}"
                            )
                            nc.scalar.activation(part[:], o_ps[:], AF.Copy)
                            parts.append(part)
                        else:
                            ob = sb.tile([128, 512], F32, tag="ob", bufs=3)
                            nc.vector.tensor_add(ob[:], o_ps[:], parts[hid_t][:])
                            nc.sync.dma_start(
                                outT_e[hid_t * 128 : (hid_t + 1) * 128, :], ob[:]
                            )

    nc.compile()
    return nc


def _prep(hidden_states, sin_table, cos_table, Wq, Wk, Wv, Wo):
    bf = ml_dtypes.bfloat16
    flat = np.asarray(hidden_states, np.float32).reshape(SG, HID)
    hT = np.ascontiguousarray(flat.T)  # [HID, SG]
    # pretile to [p, sc, kt, s]: per-partition 16KB contiguous chunk loads
    hT_t = np.ascontiguousarray(
        hT.reshape(NKT, 128, NSC, 512).transpose(1, 2, 0, 3)
    ).reshape(128, NSC * NKT * 512).astype(bf)

    cosT = np.asarray(cos_table, np.float32)[:, :64].T  # [64, S]
    sinT = np.asarray(sin_table, np.float32)[:, :64].T
    c_dup = np.tile(np.concatenate([cosT, cosT], 0), (1, B)).astype(bf)
    # sign-folded: rotate-half becomes a plain partition swap
    s_dup = np.tile(np.concatenate([-sinT, sinT], 0), (1, B)).astype(bf)

    ident = np.eye(D, dtype=np.float32).astype(bf)

    kk = np.arange(128)[:, None]
    qq = np.arange(1024)[None, :]
    masks = (kk <= qq).astype(np.float32).astype(bf)

    scale = np.float32(1.0 / np.sqrt(D))
    Wq = np.asarray(Wq, np.float32) * scale
    Wk = np.asarray(Wk, np.float32)
    Wv = np.asarray(Wv, np.float32)
    Wo = np.asarray(Wo, np.float32)

    def tile_w(w):  # [HID, F] -> [p, kt, F] flattened
        f = w.shape[1]
        return np.ascontiguousarray(
            w.reshape(NKT, 128, f).transpose(1, 0, 2)
        ).reshape(128, NKT * f).astype(bf)

    # Permute Wo rows into the order o_proj consumes the A2A output blocks:
    # a2a1 blocks: (r, h in {0,1}); a2a2 blocks: (r, h in {2,3}); then
    # pretile to [p, half, hid_t, ft, c] for 4KB-contiguous wo_t loads.
    Wo_b = Wo.reshape(H, D, HID)
    order = [4 * r + h for r in range(8) for h in (0, 1)] + [
        4 * r + h for r in range(8) for h in (2, 3)
    ]
    Wo_perm = Wo_b[order].reshape(H * D, HID)
    Wo_t = np.ascontiguousarray(
        Wo_perm.reshape(2, 16, 128, NKT, 128).transpose(2, 0, 3, 1, 4)
    ).reshape(128, 2 * NKT * 16 * 128).astype(bf)

    in_maps = []
    for c in range(N_CORES):
        in_maps.append(
            {
                "hT": hT_t,
                "wq": tile_w(Wq[:, c * 512 : (c + 1) * 512]),
                "wk": tile_w(Wk[:, c * D : (c + 1) * D]),
                "wv": tile_w(Wv[:, c * D : (c + 1) * D]),
                "wo": Wo_t,
                "c_dup": c_dup,
                "s_dup": s_dup,
                "ident": ident,
                "masks": masks,
            }
        )
    return in_maps


def kernel(**inputs) -> np.ndarray:
    global LAST_EXEC_NS, LAST_RESULT
    if "nc" not in _CACHE:
        _CACHE["nc"] = _build()
    nc = _CACHE["nc"]

    extra = {}
    if os.environ.get("BASS_TMPDIR"):
        extra["tmpdir"] = os.environ["BASS_TMPDIR"]
    if os.environ.get("BASS_TRACE_CORES"):
        extra["trace_cores"] = [
            int(c) for c in os.environ["BASS_TRACE_CORES"].split(",")
        ]
    in_maps = _prep(**inputs)
    res = run_bass_kernel_spmd(
        nc,
        in_maps,
        core_ids=list(range(N_CORES)),
        trace=bool(os.environ.get("BASS_TRACE")),
        **extra,
    )
    LAST_EXEC_NS = res.exec_time_ns
    LAST_RESULT = res

    outT = np.concatenate(
        [np.asarray(res.results[c]["outT"], np.float32) for c in range(N_CORES)],
        axis=1,
    )  # [HID, SG]
    return np.ascontiguousarray(outT.T).reshape(B, S, HID)


# revision 11
# speedup vs baseline: 1.0301x; 1.0301x over previous
"""GQA causal attention (B=2, S=2048, HID=2048, H=32, HKV=8, D=128) on 8 TRN2
NeuronCores.

Sharding: tensor-parallel over heads for QKV+attention (core c owns kv head c
and q heads 4c..4c+3), then an AllToAll switches to sequence-parallel for
o_proj (core c computes the full hidden dim for global s-chunk c). The A2A
moves 8x less data than an AllGather and needs no per-core dynamic slicing.
It is split into two collectives (head pairs) so comm overlaps attention
compute of the remaining heads and the first half of o_proj.

Device pipeline (bf16 compute, fp32 PSUM accumulation):
  1. Feature-major projections: Q^T/K^T/V^T = W^T h^T, h^T streamed. All
     HBM-resident operands are host-pretiled so every DMA is >=2KB-contiguous
     per partition (h^T chunks 16KB, weights 4-16KB) - the 1KB-line layouts
     capped DMA at ~230GB/s and starved the PE ramp.
  2. RoPE as  x*cos_dup + swap_halves(x)*sin_signed  - the rotate-half is a
     pure partition swap done by idle gpsimd SWDGE DMAs (the sign lives in the
     host-prepared sin table); cross-partition DVE ops are illegal.
  3. Transposed flash attention processed in PAIRS of 128-k-tiles: each score
     pair is one [128,1024] 2-bank PSUM tile (two matmuls), one 1024-wide exp
     on ScalarE (halves ACT instruction overhead - ACT is the binding engine
     in this phase), causal 0/1 mask on diagonal tiles, denominator via
     ones-matmul on pair-sums (quad = add of two pairs: one 1024-wide + one
     512-wide DVE add instead of three 512 adds), out^T += V_tile.T @ P^T.
  4. Two AllToAlls (heads 0-1, then 2-3) exchange attn-out^T blocks.
  5. o_proj: out^T[hid, my_s_chunk] accumulated over all 32 feature tiles
     (Wo host-pretiled into [p][half][hid_t][ft][c] so each wo_t DMA is
     4KB-contiguous), fp32 out.
Host reassembles the 8 sequence chunks and transposes back.
"""

import os

import numpy as np
import ml_dtypes

from concourse import bacc, mybir
import concourse.tile as tile
from concourse.bass_utils import run_bass_kernel_spmd

N_CORES = 8
B, S, HID = 2, 2048, 2048
H, HKV, D = 32, 8, 128
QH = H // HKV          # q heads per core
SG = B * S             # 4096 global sequence
NSC = SG // 512        # 8 s-chunks of 512
NKT = HID // 128       # 16 hid k-tiles
NFT = (H * D) // 128   # 32 o_proj contraction tiles

BF = mybir.dt.bfloat16
F32 = mybir.dt.float32
AF = mybir.ActivationFunctionType

_CACHE = {}
LAST_EXEC_NS = None
LAST_RESULT = None


def _build():
    nc = bacc.Bacc("TRN2", num_devices=N_CORES)

    # Host-pretiled layouts: partition dim first, per-partition runs contiguous.
    hT_e = nc.declare_dram_parameter("hT", [128, NSC * NKT * 512], BF, isOutput=False)
    wq_e = nc.declare_dram_parameter("wq", [128, NKT * 512], BF, isOutput=False)
    wk_e = nc.declare_dram_parameter("wk", [128, NKT * D], BF, isOutput=False)
    wv_e = nc.declare_dram_parameter("wv", [128, NKT * D], BF, isOutput=False)
    wo_e = nc.declare_dram_parameter("wo", [128, 2 * NKT * 16 * 128], BF, isOutput=False)
    cd_e = nc.declare_dram_parameter("c_dup", [D, SG], BF, isOutput=False)
    sd_e = nc.declare_dram_parameter("s_dup", [D, SG], BF, isOutput=False)
    rt_e = nc.declare_dram_parameter("rT", [D, D], BF, isOutput=False)
    id_e = nc.declare_dram_parameter("ident", [D, D], BF, isOutput=False)
    mk_e = nc.declare_dram_parameter("masks", [128, 1024], BF, isOutput=False)
    outT_e = nc.declare_dram_parameter("outT", [HID, 512], F32, isOutput=True)

    hT_r = hT_e[:].rearrange("p (sc kt s) -> p sc kt s", sc=NSC, kt=NKT)
    wq_r = wq_e[:].rearrange("p (kt f) -> p kt f", kt=NKT)
    wo_r = wo_e[:].rearrange("p (hf ht ft c) -> p hf ht ft c", hf=2, ht=NKT, ft=16)

    with tile.TileContext(nc) as tc:
        with (
            tc.tile_pool(name="cst", bufs=1) as cst,
            tc.tile_pool(name="sb", bufs=2) as sb,
            tc.tile_pool(name="ps", bufs=2, space="PSUM") as ps,
            tc.tile_pool(name="psacc", bufs=2, space="PSUM") as psacc,
            tc.tile_pool(name="dram", bufs=1, space="DRAM") as dram,
        ):
            tril = cst.tile([128, 1024], BF, tag="tril")
            nc.sync.dma_start(tril[:], mk_e[:])
            ones_mat = cst.tile([128, 128], BF, tag="ones_mat")
            nc.gpsimd.memset(ones_mat[:], 1.0)
            # partition-swap permutation for rotate-half (sign lives in s_dup)
            p_swap = cst.tile([D, D], BF, tag="p_swap")
            nc.sync.dma_start(p_swap[:], rt_e[:])

            qr = cst.tile([128, QH * SG], BF, tag="qr")
            kr = cst.tile([128, SG], BF, tag="kr")
            v_seq = cst.tile([128, SG], BF, tag="v_seq")

            # A2A bounce buffers: shard j = rows [j*256, (j+1)*256) =
            # (2 heads x 128d, s-chunk j's 512 cols).
            a2a_in = [
                dram.tile([8 * 256, 512], BF, name=f"a2ain{i}", tag=f"a2ain{i}")
                for i in (0, 1)
            ]
            a2a_out = [
                dram.tile([8 * 256, 512], BF, name=f"a2aout{i}", tag=f"a2aout{i}")
                for i in (0, 1)
            ]

            # ---- phase 1: projections + rope + V transpose ----
            with tc.tile_pool(name="p1", bufs=1) as p1, \
                 tc.tile_pool(name="htp", bufs=3) as htp:
                # wq and the first h^T chunk interleaved FIRST (4 pieces each,
                # 4KB runs) so the first projection chain streams as pieces
                # land; everything else follows in need-order.
                wq_sb = p1.tile([128, NKT, QH * D], BF, tag="wq_sb")
                ht0 = htp.tile([128, NKT, 512], BF, tag="ht")
                for kq in range(4):
                    nc.sync.dma_start(
                        wq_sb[:, 4 * kq : 4 * kq + 4, :],
                        wq_r[:, 4 * kq : 4 * kq + 4, :],
                    )
                    nc.sync.dma_start(
                        ht0[:, 4 * kq : 4 * kq + 4, :],
                        hT_r[:, 0, 4 * kq : 4 * kq + 4, :],
                    )
                ident = p1.tile([D, D], BF, tag="ident")
                nc.sync.dma_start(ident[:], id_e[:])
                wk_sb = p1.tile([128, NKT, D], BF, tag="wk_sb")
                nc.sync.dma_start(
                    wk_sb[:], wk_e[:].rearrange("p (kt f) -> p kt f", kt=NKT)
                )
                wv_sb = p1.tile([128, NKT, D], BF, tag="wv_sb")
                nc.sync.dma_start(
                    wv_sb[:], wv_e[:].rearrange("p (kt f) -> p kt f", kt=NKT)
                )
                ht1 = htp.tile([128, NKT, 512], BF, tag="ht")
                nc.sync.dma_start(ht1[:], hT_r[:, 1, :, :])
                c_d = p1.tile([D, SG], BF, tag="c_d")
                nc.sync.dma_start(c_d[:], cd_e[:])
                s_d = p1.tile([D, SG], BF, tag="s_d")
                nc.sync.dma_start(s_d[:], sd_e[:])

                # rope/V-transpose for tile i are emitted AFTER projection
                # chain i+1 so their PE ops never wait on the ACT evacuation.
                def finish_tile(sc, ft, xb):
                    if ft < QH + 1:  # rope for q heads and k
                        # rotate-half = partition swap via a PE matmul against
                        # the permutation stationary (SWDGE swap DMAs had
                        # multi-us completion latency on this critical chain)
                        rot = ps.tile([128, 512], F32, tag="mmp")
                        nc.tensor.matmul(rot[:], p_swap[:], xb[:])
                        if ft < QH:
                            dest = qr[
                                :, ft * SG + sc * 512 : ft * SG + sc * 512 + 512
                            ]
                        else:
                            dest = kr[:, sc * 512 : sc * 512 + 512]
                        cs = c_d[:, sc * 512 : (sc + 1) * 512]
                        ss = s_d[:, sc * 512 : (sc + 1) * 512]
                        nc.vector.tensor_mul(dest, xb[:], cs)
                        rtmp = sb.tile([128, 512], BF, tag="rtmp")
                        nc.vector.tensor_mul(rtmp[:], rot[:], ss)
                        nc.vector.tensor_add(dest, dest, rtmp[:])
                    else:  # v: transpose to seq-major
                        for j in range(4):
                            tp = ps.tile([128, 1024], BF, tag="mmp")
                            nc.tensor.transpose(
                                tp[:, 0:128], xb[:, j * 128 : (j + 1) * 128],
                                ident[:],
                            )
                            g = sc * 4 + j
                            nc.vector.tensor_copy(
                                v_seq[:, g * 128 : (g + 1) * 128], tp[:, 0:128]
                            )

                with nc.named_scope("proj"):
                    pending = None
                    for sc in range(NSC):
                        if sc == 0:
                            ht = ht0
                        elif sc == 1:
                            ht = ht1
                        else:
                            ht = htp.tile([128, NKT, 512], BF, tag="ht")
                            nc.sync.dma_start(ht[:], hT_r[:, sc, :, :])
                        for ft in range(QH + 2):  # 0..3 q heads, 4 k, 5 v
                            acc = ps.tile([128, 512], F32, tag="mm")
                            for kt in range(NKT):
                                if ft < QH:
                                    lhsT = wq_sb[:, kt, ft * D : (ft + 1) * D]
                                elif ft == QH:
                                    lhsT = wk_sb[:, kt, :]
                                else:
                                    lhsT = wv_sb[:, kt, :]
                                nc.tensor.matmul(
                                    acc[:], lhsT, ht[:, kt, :],
                                    start=(kt == 0), stop=(kt == NKT - 1),
                                )
                            xb = sb.tile([128, 512], BF, tag="xb", bufs=4)
                            nc.scalar.activation(xb[:], acc[:], AF.Copy)
                            if pending is not None:
                                finish_tile(*pending)
                            pending = (sc, ft, xb)
                    finish_tile(*pending)

            # ---- phase 2: attention, software-pipelined ACROSS (h,b,qc)
            # units so the score->exp->PV pipe never drains at unit
            # boundaries; epilogues (recip/ao/DMA) are deferred one pair so
            # they don't block the strict DVE FIFO ahead of the next unit's
            # mask/PV chain.
            with nc.named_scope("attn"):
                for half in range(2):
                    units = [
                        (h, b, qc)
                        for h in (2 * half, 2 * half + 1)
                        for b in range(B)
                        for qc in range(4)
                    ]
                    flat = [
                        (u, p)
                        for u, (h, b, qc) in enumerate(units)
                        for p in range(2 * qc + 2)
                    ]
                    state = {}

                    def ustate(u):
                        h, b, qc = units[u]
                        if u not in state:
                            state[u] = {
                                "acc": psacc.tile(
                                    [128, 512], F32, tag="acc",
                                    name=f"acc{half}_{u}",
                                ),
                                "den": ps.tile(
                                    [128, 512], F32, tag="mm",
                                    name=f"den{half}_{u}",
                                ),
                                "hold": [],
                            }
                        return state[u]

                    def score_pair(u, p):
                        h, b, qc = units[u]
                        qs = h * SG + b * S + qc * 512
                        s_ps = ps.tile(
                            [128, 1024], F32, tag="mmp",
                            name=f"s{half}_{u}_{p}",
                        )
                        for hf in range(2):
                            kt = 2 * p + hf
                            j = kt - 4 * qc
                            o = j * 128 if j > 0 else 0
                            nc.tensor.matmul(
                                s_ps[:, hf * 512 + o : (hf + 1) * 512],
                                kr[:, b * S + kt * 128 : b * S + (kt + 1) * 128],
                                qr[:, qs + o : qs + 512],
                            )
                        return s_ps

                    def consume(u, p, s_ps):
                        h, b, qc = units[u]
                        st = ustate(u)
                        nkt = 4 * qc + 4
                        acc, den = st["acc"], st["den"]

                        def qoff(kt):
                            j = kt - 4 * qc
                            return j * 128 if j > 0 else 0

                        offdiag = 2 * p + 1 < 4 * qc
                        pT = sb.tile([128, 1024], BF, tag="pT", bufs=4)
                        if offdiag:
                            nc.scalar.activation(pT[:], s_ps[:], AF.Exp)
                        else:
                            for hf in range(2):
                                kt = 2 * p + hf
                                o = qoff(kt)
                                sl = slice(hf * 512 + o, (hf + 1) * 512)
                                nc.scalar.activation(
                                    pT[:, sl], s_ps[:, sl], AF.Exp
                                )
                                if kt - 4 * qc >= 0:
                                    nc.vector.tensor_mul(
                                        pT[:, sl], pT[:, sl], tril[:, : 512 - o]
                                    )
                        for hf in range(2):
                            kt = 2 * p + hf
                            o = qoff(kt)
                            g = b * 16 + kt
                            nc.tensor.matmul(
                                acc[:, o:512],
                                v_seq[:, g * 128 : (g + 1) * 128],
                                pT[:, hf * 512 + o : (hf + 1) * 512],
                                start=(kt == 0), stop=(kt == nkt - 1),
                            )
                        if offdiag:
                            # den is linear: quad = pair+pair via one 1024-wide
                            # and one 512 DVE add -> 1 den matmul per 4 tiles.
                            st["hold"].append(pT)
                            if len(st["hold"]) == 2:
                                quad = sb.tile(
                                    [128, 1024], BF, tag="quad", bufs=2
                                )
                                nc.vector.tensor_add(
                                    quad[:], st["hold"][0][:], st["hold"][1][:]
                                )
                                qsum = sb.tile(
                                    [128, 512], BF, tag="qsum", bufs=2
                                )
                                nc.vector.tensor_add(
                                    qsum[:], quad[:, 0:512], quad[:, 512:1024]
                                )
                                nc.tensor.matmul(
                                    den[:], ones_mat[:], qsum[:],
                                    start=(p == 1), stop=False,
                                )
                                st["hold"] = []
                        else:
                            for hf in range(2):
                                kt = 2 * p + hf
                                o = qoff(kt)
                                nc.tensor.matmul(
                                    den[:, o:512],
                                    ones_mat[:],
                                    pT[:, hf * 512 + o : (hf + 1) * 512],
                                    start=(kt == 0), stop=(kt == nkt - 1),
                                )

                    def epilogue(u):
                        h, b, qc = units[u]
                        st = state.pop(u)
                        # den rows are identical (all-ones stationary) ==
                        # denominator already broadcast across partitions.
                        rb_sb = sb.tile([128, 512], F32, tag="rb_sb")
                        nc.vector.reciprocal_approx_fast(rb_sb[:], st["den"][:])
                        ao = sb.tile([128, 512], BF, tag="ao", bufs=3)
                        nc.vector.tensor_mul(ao[:], st["acc"][:], rb_sb[:])
                        hh = h % 2
                        sc = b * 4 + qc
                        nc.sync.dma_start(
                            a2a_in[half][
                                sc * 256 + hh * 128 : sc * 256 + (hh + 1) * 128, :
                            ],
                            ao[:],
                        )

                    LOOK = 2
                    pipe = [score_pair(*flat[i]) for i in range(LOOK)]
                    pend = None
                    for i, (u, p) in enumerate(flat):
                        if i + LOOK < len(flat):
                            pipe.append(score_pair(*flat[i + LOOK]))
                        consume(u, p, pipe.pop(0))
                        if pend is not None and pend != u:
                            epilogue(pend)
                            pend = None
                        if p == 2 * units[u][2] + 1:  # last pair of unit
                            pend = u
                    if pend is not None:
                        epilogue(pend)
                    nc.gpsimd.collective_compute(
                        "AllToAll",
                        mybir.AluOpType.bypass,
                        replica_groups=[list(range(N_CORES))],
                        ins=[a2a_in[half].opt()],
                        outs=[a2a_out[half].opt()],
                    )

            # ---- phase 4: o_proj for my s-chunk, all hidden columns.
            # Two passes: pass 0 (features from A2A1) accumulates to SBUF
            # partials while A2A2 is still in flight; pass 1 adds the rest.
            with nc.named_scope("oproj"), \
                 tc.tile_pool(name="agp", bufs=1) as agp, \
                 tc.tile_pool(name="wop", bufs=8) as wop, \
                 tc.tile_pool(name="prt", bufs=1) as prt:
                parts = []
                for half in range(2):
                    agt = agp.tile([128, 16, 512], BF, tag=f"ag{half}")
                    # gpsimd (SWDGE) queue: serialized behind the collective
                    # wait anyway - keeps this collective-gated load off the
                    # SP HWDGE queue. 4 coarse pieces instead of 16 singles
                    # so the first o_proj chain isn't gated on SWDGE dispatch.
                    for fq in range(4):
                        nc.gpsimd.dma_start(
                            agt[:, 4 * fq : 4 * fq + 4, :],
                            a2a_out[half][
                                4 * fq * 128 : (4 * fq + 4) * 128, :
                            ].rearrange("(ft p) s -> p ft s", p=128),
                        )
                    for hid_t in range(NKT):  # 16 tiles of 128 hidden cols
                        wo_t = wop.tile([128, 16, 128], BF, tag="wo_t")
                        nc.scalar.dma_start(wo_t[:], wo_r[:, half, hid_t, :, :])
                        o_ps = ps.tile([128, 512], F32, tag="mm")
                        for ft in range(16):
                            nc.tensor.matmul(
                                o_ps[:],
                                wo_t[:, ft, :],
                                agt[:, ft, :],
                                start=(ft == 0),
                                stop=(ft == 15),
                            )
                        if half == 0:
                            part = prt.tile(
                                [128, 512], F32, tag=f"part{hid_t}"
                            )
                            nc.scalar.activation(part[:], o_ps[:], AF.Copy)
                            parts.append(part)
                        else:
                            ob = sb.tile([128, 512], F32, tag="ob", bufs=3)
                            nc.vector.tensor_add(ob[:], o_ps[:], parts[hid_t][:])
                            nc.sync.dma_start(
                                outT_e[hid_t * 128 : (hid_t + 1) * 128, :], ob[:]
                            )

    nc.compile()
    return nc


def _prep(hidden_states, sin_table, cos_table, Wq, Wk, Wv, Wo):
    bf = ml_dtypes.bfloat16
    flat = np.asarray(hidden_states, np.float32).reshape(SG, HID)
    hT = np.ascontiguousarray(flat.T)  # [HID, SG]
    # pretile to [p, sc, kt, s]: per-partition 16KB contiguous chunk loads
    hT_t = np.ascontiguousarray(
        hT.reshape(NKT, 128, NSC, 512).transpose(1, 2, 0, 3)
    ).reshape(128, NSC * NKT * 512).astype(bf)

    cosT = np.asarray(cos_table, np.float32)[:, :64].T  # [64, S]
    sinT = np.asarray(sin_table, np.float32)[:, :64].T
    c_dup = np.tile(np.concatenate([cosT, cosT], 0), (1, B)).astype(bf)
    # sign-folded: rotate-half becomes a plain partition swap
    s_dup = np.tile(np.concatenate([-sinT, sinT], 0), (1, B)).astype(bf)

    ident = np.eye(D, dtype=np.float32).astype(bf)
    # unsigned half-swap permutation (symmetric, so P^T = P)
    P = np.zeros((D, D), np.float32)
    for i in range(64):
        P[i, i + 64] = 1.0
        P[i + 64, i] = 1.0
    p_swap = P.astype(bf)

    kk = np.arange(128)[:, None]
    qq = np.arange(1024)[None, :]
    masks = (kk <= qq).astype(np.float32).astype(bf)

    scale = np.float32(1.0 / np.sqrt(D))
    Wq = np.asarray(Wq, np.float32) * scale
    Wk = np.asarray(Wk, np.float32)
    Wv = np.asarray(Wv, np.float32)
    Wo = np.asarray(Wo, np.float32)

    def tile_w(w):  # [HID, F] -> [p, kt, F] flattened
        f = w.shape[1]
        return np.ascontiguousarray(
            w.reshape(NKT, 128, f).transpose(1, 0, 2)
        ).reshape(128, NKT * f).astype(bf)

    # Permute Wo rows into the order o_proj consumes the A2A output blocks:
    # a2a1 blocks: (r, h in {0,1}); a2a2 blocks: (r, h in {2,3}); then
    # pretile to [p, half, hid_t, ft, c] for 4KB-contiguous wo_t loads.
    Wo_b = Wo.reshape(H, D, HID)
    order = [4 * r + h for r in range(8) for h in (0, 1)] + [
        4 * r + h for r in range(8) for h in (2, 3)
    ]
    Wo_perm = Wo_b[order].reshape(H * D, HID)
    Wo_t = np.ascontiguousarray(
        Wo_perm.reshape(2, 16, 128, NKT, 128).transpose(2, 0, 3, 1, 4)
    ).reshape(128, 2 * NKT * 16 * 128).astype(bf)

    in_maps = []
    for c in range(N_CORES):
        in_maps.append(
            {
                "hT": hT_t,
                "wq": tile_w(Wq[:, c * 512 : (c + 1) * 512]),
                "wk": tile_w(Wk[:, c * D : (c + 1) * D]),
                "wv": tile_w(Wv[:, c * D : (c + 1) * D]),
                "wo": Wo_t,
                "c_dup": c_dup,
                "s_dup": s_dup,
                "rT": p_swap,
                "ident": ident,
                "masks": masks,
            }
        )
    return in_maps


def kernel(**inputs) -> np.ndarray:
    global LAST_EXEC_NS, LAST_RESULT
    if "nc" not in _CACHE:
        _CACHE["nc"] = _build()
    nc = _CACHE["nc"]

    extra = {}
    if os.environ.get("BASS_TMPDIR"):
        extra["tmpdir"] = os.environ["BASS_TMPDIR"]
    if os.environ.get("BASS_TRACE_CORES"):
        extra["trace_cores"] = [
            int(c) for c in os.environ["BASS_TRACE_CORES"].split(",")
        ]
    in_maps = _prep(**inputs)
    res = run_bass_kernel_spmd(
        nc,
        in_maps,
        core_ids=list(range(N_CORES)),
        trace=bool(os.environ.get("BASS_TRACE")),
        **extra,
    )
    LAST_EXEC_NS = res.exec_time_ns
    LAST_RESULT = res

    outT = np.concatenate(
        [np.asarray(res.results[c]["outT"], np.float32) for c in range(N_CORES)],
        axis=1,
    )  # [HID, SG]
    return np.ascontiguousarray(outT.T).reshape(B, S, HID)


# revision 24
# speedup vs baseline: 1.0523x; 1.0216x over previous
"""GQA causal attention (B=2, S=2048, HID=2048, H=32, HKV=8, D=128) on 8 TRN2
NeuronCores.

Sharding: tensor-parallel over heads for QKV+attention (core c owns kv head c
and q heads 4c..4c+3), then an AllToAll switches to sequence-parallel for
o_proj (core c computes the full hidden dim for global s-chunk c). The A2A
moves 8x less data than an AllGather and needs no per-core dynamic slicing.
It is split into two collectives (head pairs) so comm overlaps attention
compute of the remaining heads and the first half of o_proj.

Device pipeline (bf16 compute, fp32 PSUM accumulation):
  1. Feature-major projections: Q^T/K^T/V^T = W^T h^T, h^T streamed. All
     HBM-resident operands are host-pretiled so every DMA is >=2KB-contiguous
     per partition (h^T chunks 16KB, weights 4-16KB) - the 1KB-line layouts
     capped DMA at ~230GB/s and starved the PE ramp.
  2. RoPE as  x*cos_dup + swap_halves(x)*sin_signed  - the rotate-half is a
     pure partition swap done by idle gpsimd SWDGE DMAs (the sign lives in the
     host-prepared sin table); cross-partition DVE ops are illegal.
  3. Transposed flash attention processed in PAIRS of 128-k-tiles: each score
     pair is one [128,1024] 2-bank PSUM tile (two matmuls), one 1024-wide exp
     on ScalarE (halves ACT instruction overhead - ACT is the binding engine
     in this phase), causal 0/1 mask on diagonal tiles, denominator via
     ones-matmul on pair-sums (quad = add of two pairs: one 1024-wide + one
     512-wide DVE add instead of three 512 adds), out^T += V_tile.T @ P^T.
  4. Two AllToAlls (heads 0-1, then 2-3) exchange attn-out^T blocks.
  5. o_proj: out^T[hid, my_s_chunk] accumulated over all 32 feature tiles
     (Wo host-pretiled into [p][half][hid_t][ft][c] so each wo_t DMA is
     4KB-contiguous), fp32 out.
Host reassembles the 8 sequence chunks and transposes back.
"""

import os

import numpy as np
import ml_dtypes

from concourse import bacc, mybir
import concourse.tile as tile
from concourse.bass_utils import run_bass_kernel_spmd

N_CORES = 8
B, S, HID = 2, 2048, 2048
H, HKV, D = 32, 8, 128
QH = H // HKV          # q heads per core
SG = B * S             # 4096 global sequence
NSC = SG // 512        # 8 s-chunks of 512
NKT = HID // 128       # 16 hid k-tiles
NFT = (H * D) // 128   # 32 o_proj contraction tiles

BF = mybir.dt.bfloat16
F32 = mybir.dt.float32
I32 = mybir.dt.int32
AF = mybir.ActivationFunctionType
ALU = mybir.AluOpType

# Schraudolph fast-exp: exp(x) ~= bitcast_f32(int32(A*x + B)). ~3% max
# per-element error that largely cancels through softmax normalization;
# used on a few pairs per unit to offload the saturated ScalarE onto DVE.
EXP_A = 12102203.1616  # 2^23 / ln 2
EXP_B = 1065353216.0 - 366392.0  # 127 * 2^23 - C

_CACHE = {}
LAST_EXEC_NS = None
LAST_RESULT = None


def _build():
    nc = bacc.Bacc("TRN2", num_devices=N_CORES)

    # Host-pretiled layouts: partition dim first, per-partition runs contiguous.
    hT_e = nc.declare_dram_parameter("hT", [128, NSC * NKT * 512], BF, isOutput=False)
    wq_e = nc.declare_dram_parameter("wq", [128, NKT * 512], BF, isOutput=False)
    wk_e = nc.declare_dram_parameter("wk", [128, NKT * D], BF, isOutput=False)
    wv_e = nc.declare_dram_parameter("wv", [128, NKT * D], BF, isOutput=False)
    wo_e = nc.declare_dram_parameter("wo", [128, 2 * NKT * 16 * 128], BF, isOutput=False)
    cd_e = nc.declare_dram_parameter("c_dup", [D, SG], BF, isOutput=False)
    sd_e = nc.declare_dram_parameter("s_dup", [D, SG], BF, isOutput=False)
    id_e = nc.declare_dram_parameter("ident", [D, D], BF, isOutput=False)
    mk_e = nc.declare_dram_parameter("masks", [128, 1024], BF, isOutput=False)
    outT_e = nc.declare_dram_parameter("outT", [HID, 512], F32, isOutput=True)

    hT_r = hT_e[:].rearrange("p (sc kt s) -> p sc kt s", sc=NSC, kt=NKT)
    wq_r = wq_e[:].rearrange("p (kt f) -> p kt f", kt=NKT)
    wo_r = wo_e[:].rearrange("p (hf ht ft c) -> p hf ht ft c", hf=2, ht=NKT, ft=16)

    with tile.TileContext(nc) as tc:
        with (
            tc.tile_pool(name="cst", bufs=1) as cst,
            tc.tile_pool(name="sb", bufs=2) as sb,
            tc.tile_pool(name="ps", bufs=2, space="PSUM") as ps,
            tc.tile_pool(name="psacc", bufs=2, space="PSUM") as psacc,
            tc.tile_pool(name="dram", bufs=1, space="DRAM") as dram,
        ):
            ones_mat = cst.tile([128, 128], BF, tag="ones_mat")
            nc.gpsimd.memset(ones_mat[:], 1.0)

            qr = cst.tile([128, QH * SG], BF, tag="qr")
            kr = cst.tile([128, SG], BF, tag="kr")
            v_seq = cst.tile([128, SG], BF, tag="v_seq")

            # A2A bounce buffers: shard j = rows [j*256, (j+1)*256) =
            # (2 heads x 128d, s-chunk j's 512 cols).
            a2a_in = [
                dram.tile([8 * 256, 512], BF, name=f"a2ain{i}", tag=f"a2ain{i}")
                for i in (0, 1)
            ]
            a2a_out = [
                dram.tile([8 * 256, 512], BF, name=f"a2aout{i}", tag=f"a2aout{i}")
                for i in (0, 1)
            ]

            # ---- phase 1: projections + rope + V transpose ----
            with tc.tile_pool(name="p1", bufs=1) as p1, \
                 tc.tile_pool(name="htp", bufs=3) as htp:
                # wq and the first h^T chunk interleaved FIRST (wq on the SP
                # HWDGE queue, h^T on the ACT one so their issue doesn't
                # serialize; small leading pieces so the first chain starts
                # on kt 0-1 while the rest streams). Tables follow in
                # need-order.
                wq_sb = p1.tile([128, NKT, QH * D], BF, tag="wq_sb")
                ht0 = htp.tile([128, NKT, 512], BF, tag="ht")
                for lo, hi in ((0, 2), (2, 6), (6, 10), (10, 14), (14, 16)):
                    nc.sync.dma_start(
                        wq_sb[:, lo:hi, :], wq_r[:, lo:hi, :]
                    )
                    nc.scalar.dma_start(
                        ht0[:, lo:hi, :], hT_r[:, 0, lo:hi, :]
                    )
                tril = cst.tile([128, 1024], BF, tag="tril")
                nc.sync.dma_start(tril[:], mk_e[:])
                ident = p1.tile([D, D], BF, tag="ident")
                nc.sync.dma_start(ident[:], id_e[:])
                wk_sb = p1.tile([128, NKT, D], BF, tag="wk_sb")
                nc.sync.dma_start(
                    wk_sb[:], wk_e[:].rearrange("p (kt f) -> p kt f", kt=NKT)
                )
                wv_sb = p1.tile([128, NKT, D], BF, tag="wv_sb")
                nc.sync.dma_start(
                    wv_sb[:], wv_e[:].rearrange("p (kt f) -> p kt f", kt=NKT)
                )
                ht1 = htp.tile([128, NKT, 512], BF, tag="ht")
                nc.scalar.dma_start(ht1[:], hT_r[:, 1, :, :])
                c_d = p1.tile([D, SG], BF, tag="c_d")
                nc.sync.dma_start(c_d[:], cd_e[:])
                s_d = p1.tile([D, SG], BF, tag="s_d")
                nc.sync.dma_start(s_d[:], sd_e[:])

                # rope/V-transpose for tile i are emitted AFTER projection
                # chain i+1 so their PE ops never wait on the ACT evacuation.
                # rope feature pairs are host-interleaved to (even, odd)
                # positions (scores are invariant: q and k share the
                # permutation), so rotate-half is an intra-quadrant even/odd
                # partition swap - a single DVE stream_shuffle.
                swap_mask = [i ^ 1 for i in range(32)]

                def finish_tile(sc, ft, xb):
                    if ft < QH + 1:  # rope for q heads and k
                        sh = sb.tile([128, 512], BF, tag="sh", bufs=3)
                        nc.vector.stream_shuffle(sh[:], xb[:], swap_mask)
                        if ft < QH:
                            dest = qr[
                                :, ft * SG + sc * 512 : ft * SG + sc * 512 + 512
                            ]
                        else:
                            dest = kr[:, sc * 512 : sc * 512 + 512]
                        cs = c_d[:, sc * 512 : (sc + 1) * 512]
                        ss = s_d[:, sc * 512 : (sc + 1) * 512]
                        nc.vector.tensor_mul(dest, xb[:], cs)
                        rtmp = sb.tile([128, 512], BF, tag="rtmp")
                        nc.vector.tensor_mul(rtmp[:], sh[:], ss)
                        nc.vector.tensor_add(dest, dest, rtmp[:])
                    else:  # v: transpose to seq-major
                        for j in range(4):
                            tp = ps.tile([128, 1024], BF, tag="mmp")
                            nc.tensor.transpose(
                                tp[:, 0:128], xb[:, j * 128 : (j + 1) * 128],
                                ident[:],
                            )
                            g = sc * 4 + j
                            nc.vector.tensor_copy(
                                v_seq[:, g * 128 : (g + 1) * 128], tp[:, 0:128]
                            )

                with nc.named_scope("proj"):
                    pending = None
                    for sc in range(NSC):
                        if sc == 0:
                            ht = ht0
                        elif sc == 1:
                            ht = ht1
                        else:
                            ht = htp.tile([128, NKT, 512], BF, tag="ht")
                            nc.sync.dma_start(ht[:], hT_r[:, sc, :, :])
                        for ft in range(QH + 2):  # 0..3 q heads, 4 k, 5 v
                            acc = ps.tile([128, 512], F32, tag="mm")
                            for kt in range(NKT):
                                if ft < QH:
                                    lhsT = wq_sb[:, kt, ft * D : (ft + 1) * D]
                                elif ft == QH:
                                    lhsT = wk_sb[:, kt, :]
                                else:
                                    lhsT = wv_sb[:, kt, :]
                                nc.tensor.matmul(
                                    acc[:], lhsT, ht[:, kt, :],
                                    start=(kt == 0), stop=(kt == NKT - 1),
                                )
                            xb = sb.tile([128, 512], BF, tag="xb", bufs=4)
                            nc.scalar.activation(xb[:], acc[:], AF.Copy)
                            if pending is not None:
                                finish_tile(*pending)
                            pending = (sc, ft, xb)
                    finish_tile(*pending)

            # ---- phase 2: attention, software-pipelined ACROSS (h,b,qc)
            # units so the score->exp->PV pipe never drains at unit
            # boundaries; epilogues (recip/ao/DMA) are deferred one pair so
            # they don't block the strict DVE FIFO ahead of the next unit's
            # mask/PV chain.
            with nc.named_scope("attn"):
                for half in range(2):
                    # qc descending: the short qc=0 unit sits between long
                    # ones, so PV(u, pair0)'s wait on ao(u-2) (acc PSUM
                    # double-buffer rotation) is always covered by >=4 pairs
                    # of preceding PE work.
                    units = [
                        (h, b, qc)
                        for h in (2 * half, 2 * half + 1)
                        for b in range(B)
                        for qc in (3, 2, 1, 0)
                    ]
                    flat = [
                        (u, p)
                        for u, (h, b, qc) in enumerate(units)
                        for p in range(2 * qc + 2)
                    ]
                    state = {}

                    def ustate(u):
                        h, b, qc = units[u]
                        if u not in state:
                            state[u] = {
                                "acc": psacc.tile(
                                    [128, 512], F32, tag="acc",
                                    name=f"acc{half}_{u}",
                                ),
                                "den": ps.tile(
                                    [128, 512], F32, tag="mm",
                                    name=f"den{half}_{u}",
                                ),
                                "hold": [],
                            }
                        return state[u]

                    def score_pair(u, p):
                        h, b, qc = units[u]
                        qs = h * SG + b * S + qc * 512
                        s_ps = ps.tile(
                            [128, 1024], F32, tag="mmp",
                            name=f"s{half}_{u}_{p}",
                        )
                        for hf in range(2):
                            kt = 2 * p + hf
                            j = kt - 4 * qc
                            o = j * 128 if j > 0 else 0
                            nc.tensor.matmul(
                                s_ps[:, hf * 512 + o : (hf + 1) * 512],
                                kr[:, b * S + kt * 128 : b * S + (kt + 1) * 128],
                                qr[:, qs + o : qs + 512],
                            )
                        return s_ps

                    def consume(u, p, s_ps):
                        h, b, qc = units[u]
                        st = ustate(u)
                        nkt = 4 * qc + 4
                        acc, den = st["acc"], st["den"]

                        def qoff(kt):
                            j = kt - 4 * qc
                            return j * 128 if j > 0 else 0

                        offdiag = 2 * p + 1 < 4 * qc
                        pT = sb.tile([128, 1024], BF, tag="pT", bufs=4)
                        if offdiag:
                            nc.scalar.activation(pT[:], s_ps[:], AF.Exp)
                        else:
                            for hf in range(2):
                                kt = 2 * p + hf
                                o = qoff(kt)
                                sl = slice(hf * 512 + o, (hf + 1) * 512)
                                nc.scalar.activation(
                                    pT[:, sl], s_ps[:, sl], AF.Exp
                                )
                                if kt - 4 * qc >= 0:
                                    nc.vector.tensor_mul(
                                        pT[:, sl], pT[:, sl], tril[:, : 512 - o]
                                    )
                        for hf in range(2):
                            kt = 2 * p + hf
                            o = qoff(kt)
                            g = b * 16 + kt
                            nc.tensor.matmul(
                                acc[:, o:512],
                                v_seq[:, g * 128 : (g + 1) * 128],
                                pT[:, hf * 512 + o : (hf + 1) * 512],
                                start=(kt == 0), stop=(kt == nkt - 1),
                            )
                        if offdiag:
                            # den is linear: quad = pair+pair via one 1024-wide
                            # and one 512 DVE add -> 1 den matmul per 4 tiles.
                            st["hold"].append(pT)
                            if len(st["hold"]) == 2:
                                quad = sb.tile(
                                    [128, 1024], BF, tag="quad", bufs=2
                                )
                                nc.vector.tensor_add(
                                    quad[:], st["hold"][0][:], st["hold"][1][:]
                                )
                                qsum = sb.tile(
                                    [128, 512], BF, tag="qsum", bufs=2
                                )
                                nc.vector.tensor_add(
                                    qsum[:], quad[:, 0:512], quad[:, 512:1024]
                                )
                                nc.tensor.matmul(
                                    den[:], ones_mat[:], qsum[:],
                                    start=(p == 1), stop=False,
                                )
                                st["hold"] = []
                        else:
                            for hf in range(2):
                                kt = 2 * p + hf
                                o = qoff(kt)
                                nc.tensor.matmul(
                                    den[:, o:512],
                                    ones_mat[:],
                                    pT[:, hf * 512 + o : (hf + 1) * 512],
                                    start=(kt == 0), stop=(kt == nkt - 1),
                                )

                    def epilogue(u):
                        h, b, qc = units[u]
                        st = state.pop(u)
                        # den rows are identical (all-ones stationary) ==
                        # denominator already broadcast across partitions.
                        rb_sb = sb.tile([128, 512], F32, tag="rb_sb")
                        nc.vector.reciprocal_approx_fast(rb_sb[:], st["den"][:])
                        ao = sb.tile([128, 512], BF, tag="ao", bufs=3)
                        nc.vector.tensor_mul(ao[:], st["acc"][:], rb_sb[:])
                        hh = h % 2
                        sc = b * 4 + qc
                        nc.sync.dma_start(
                            a2a_in[half][
                                sc * 256 + hh * 128 : sc * 256 + (hh + 1) * 128, :
                            ],
                            ao[:],
                        )

                    LOOK = 2
                    pipe = [score_pair(*flat[i]) for i in range(LOOK)]
                    pend = None
                    for i, (u, p) in enumerate(flat):
                        if i + LOOK < len(flat):
                            pipe.append(score_pair(*flat[i + LOOK]))
                        consume(u, p, pipe.pop(0))
                        if pend is not None and pend != u:
                            epilogue(pend)
                            pend = None
                        if p == 2 * units[u][2] + 1:  # last pair of unit
                            pend = u
                    if pend is not None:
                        epilogue(pend)
                    nc.gpsimd.collective_compute(
                        "AllToAll",
                        mybir.AluOpType.bypass,
                        replica_groups=[list(range(N_CORES))],
                        ins=[a2a_in[half].opt()],
                        outs=[a2a_out[half].opt()],
                    )

            # ---- phase 4: o_proj for my s-chunk, all hidden columns.
            # Two passes: pass 0 (features from A2A1) accumulates to SBUF
            # partials while A2A2 is still in flight; pass 1 adds the rest.
            with nc.named_scope("oproj"), \
                 tc.tile_pool(name="agp", bufs=1) as agp, \
                 tc.tile_pool(name="wop", bufs=8) as wop, \
                 tc.tile_pool(name="prt", bufs=1) as prt:
                parts = []
                for half in range(2):
                    agt = agp.tile([128, 16, 512], BF, tag=f"ag{half}")
                    # gpsimd (SWDGE) queue: serialized behind the collective
                    # wait anyway - keeps this collective-gated load off the
                    # SP HWDGE queue. 4 coarse pieces instead of 16 singles
                    # so the first o_proj chain isn't gated on SWDGE dispatch.
                    for fq in range(4):
                        nc.gpsimd.dma_start(
                            agt[:, 4 * fq : 4 * fq + 4, :],
                            a2a_out[half][
                                4 * fq * 128 : (4 * fq + 4) * 128, :
                            ].rearrange("(ft p) s -> p ft s", p=128),
                        )
                    for hid_t in range(NKT):  # 16 tiles of 128 hidden cols
                        wo_t = wop.tile([128, 16, 128], BF, tag="wo_t")
                        nc.scalar.dma_start(wo_t[:], wo_r[:, half, hid_t, :, :])
                        o_ps = ps.tile([128, 512], F32, tag="mm")
                        for ft in range(16):
                            nc.tensor.matmul(
                                o_ps[:],
                                wo_t[:, ft, :],
                                agt[:, ft, :],
                                start=(ft == 0),
                                stop=(ft == 15),
                            )
                        if half == 0:
                            part = prt.tile(
                                [128, 512], F32, tag=f"part{hid_t}"
                            )
                            nc.scalar.activation(part[:], o_ps[:], AF.Copy)
                            parts.append(part)
                        else:
                            ob = sb.tile([128, 512], F32, tag="ob", bufs=3)
                            nc.vector.tensor_add(ob[:], o_ps[:], parts[hid_t][:])
                            nc.sync.dma_start(
                                outT_e[hid_t * 128 : (hid_t + 1) * 128, :], ob[:]
                            )

    nc.compile()
    return nc


def _prep(hidden_states, sin_table, cos_table, Wq, Wk, Wv, Wo):
    bf = ml_dtypes.bfloat16
    flat = np.asarray(hidden_states, np.float32).reshape(SG, HID)
    hT = np.ascontiguousarray(flat.T)  # [HID, SG]
    # pretile to [p, sc, kt, s]: per-partition 16KB contiguous chunk loads
    hT_t = np.ascontiguousarray(
        hT.reshape(NKT, 128, NSC, 512).transpose(1, 2, 0, 3)
    ).reshape(128, NSC * NKT * 512).astype(bf)

    cosT = np.asarray(cos_table, np.float32)[:, :64].T  # [64, S]
    sinT = np.asarray(sin_table, np.float32)[:, :64].T
    # rope features interleaved: pair k lives at rows (2k, 2k+1), so
    # rotate-half is an even/odd partition swap (sign folded into s_dup)
    c_il = np.repeat(cosT, 2, axis=0)  # [128, S]
    s_il = np.empty((D, S), np.float32)
    s_il[0::2] = -sinT
    s_il[1::2] = sinT
    c_dup = np.tile(c_il, (1, B)).astype(bf)
    s_dup = np.tile(s_il, (1, B)).astype(bf)
    # matching column permutation of each 128-wide head block of Wq/Wk:
    # new position 2k <- old k, 2k+1 <- old k+64
    il_perm = np.empty(D, np.int64)
    il_perm[0::2] = np.arange(64)
    il_perm[1::2] = np.arange(64) + 64

    ident = np.eye(D, dtype=np.float32).astype(bf)

    kk = np.arange(128)[:, None]
    qq = np.arange(1024)[None, :]
    masks = (kk <= qq).astype(np.float32).astype(bf)

    scale = np.float32(1.0 / np.sqrt(D))
    Wq = np.asarray(Wq, np.float32) * scale
    Wk = np.asarray(Wk, np.float32)
    Wv = np.asarray(Wv, np.float32)
    Wo = np.asarray(Wo, np.float32)
    # interleave rope feature pairs within every 128-wide head block
    Wq = Wq.reshape(HID, H, D)[:, :, il_perm].reshape(HID, H * D)
    Wk = Wk.reshape(HID, HKV, D)[:, :, il_perm].reshape(HID, HKV * D)

    def tile_w(w):  # [HID, F] -> [p, kt, F] flattened
        f = w.shape[1]
        return np.ascontiguousarray(
            w.reshape(NKT, 128, f).transpose(1, 0, 2)
        ).reshape(128, NKT * f).astype(bf)

    # Permute Wo rows into the order o_proj consumes the A2A output blocks:
    # a2a1 blocks: (r, h in {0,1}); a2a2 blocks: (r, h in {2,3}); then
    # pretile to [p, half, hid_t, ft, c] for 4KB-contiguous wo_t loads.
    Wo_b = Wo.reshape(H, D, HID)
    order = [4 * r + h for r in range(8) for h in (0, 1)] + [
        4 * r + h for r in range(8) for h in (2, 3)
    ]
    Wo_perm = Wo_b[order].reshape(H * D, HID)
    Wo_t = np.ascontiguousarray(
        Wo_perm.reshape(2, 16, 128, NKT, 128).transpose(2, 0, 3, 1, 4)
    ).reshape(128, 2 * NKT * 16 * 128).astype(bf)

    in_maps = []
    for c in range(N_CORES):
        in_maps.append(
            {
                "hT": hT_t,
                "wq": tile_w(Wq[:, c * 512 : (c + 1) * 512]),
                "wk": tile_w(Wk[:, c * D : (c + 1) * D]),
                "wv": tile_w(Wv[:, c * D : (c + 1) * D]),
                "wo": Wo_t,
                "c_dup": c_dup,
                "s_dup": s_dup,
                "ident": ident,
                "masks": masks,
            }
        )
    return in_maps


def kernel(**inputs) -> np.ndarray:
    global LAST_EXEC_NS, LAST_RESULT
    if "nc" not in _CACHE:
        _CACHE["nc"] = _build()
    nc = _CACHE["nc"]

    extra = {}
    if os.environ.get("BASS_TMPDIR"):
        extra["tmpdir"] = os.environ["BASS_TMPDIR"]
    if os.environ.get("BASS_TRACE_CORES"):
        extra["trace_cores"] = [
            int(c) for c in os.environ["BASS_TRACE_CORES"].split(",")
        ]
    in_maps = _prep(**inputs)
    res = run_bass_kernel_spmd(
        nc,
        in_maps,
        core_ids=list(range(N_CORES)),
        trace=bool(os.environ.get("BASS_TRACE")),
        **extra,
    )
    LAST_EXEC_NS = res.exec_time_ns
    LAST_RESULT = res

    outT = np.concatenate(
        [np.asarray(res.results[c]["outT"], np.float32) for c in range(N_CORES)],
        axis=1,
    )  # [HID, SG]
    return np.ascontiguousarray(outT.T).reshape(B, S, HID)
